# revision 7
# baseline (speedup 1.0000x reference)
"""MultiHeadGAT Bass kernel for Trainium2 (8 NeuronCores, batch-parallel).

Math (per batch b, head h):
  Wh = x @ W[h]                      (N, F_OUT)
  s1_i = Wh @ a1, s2_j = Wh @ a2     (N,)
  z[i,j] = s1_i + s2_j + ab
  exps = exp(leaky_relu(z, 0.2)) * A
  attn[i,j] = exps[i,j] / (sum_i' exps[i',j] + eps)    (softmax over dim i!)
  out = attn @ Wh; concat heads

Key identity: exp(leaky(z)) = exp(0.2 z) * max(exp(0.8 z), 1), and both
exponentials are rank-1 separable over (i, j).  With
  e5r_i = exp(0.2 (s1_i + ab)),  e5c_j = exp(0.2 s2_j),  e4c_j = exp(0.8 s2_j)
the masked field in transposed layout (j on partitions) is
  ET[j,i] = AT[j,i] * e5c_j * e5r_i * max(e5r_i^4 * e4c_j, 1)
which one custom DVE op computes per 128-row strip, bf16 out, with the
column-softmax denominator d_j = sum_i ET[j,i] accumulated for free.
TensorE then computes outT[o,i] += (Wh[j,o]/(d_j+eps)) . ET[j,i].
"""

import numpy as np
import ml_dtypes
from operator import add

import concourse.bass as bass
import concourse.bacc as bacc
import concourse.mybir as mybir
import concourse.tile as tile
import concourse.dve_ops as dve_ops_mod
from concourse.dve_spec import Spec, Src0, Src1, C0, C1, C2, sq, maxx, lower, _has_src1
from concourse.dve_uop import DveOpSpec
from concourse.bass_utils import run_bass_kernel_spmd

B, N, F_IN, F_OUT, H = 8, 1024, 128, 64, 4
EPS = 1e-7
NEG_SLOPE = 0.2
NCORES = 8
NSTRIP = N // 128  # 8 j-strips per core

F32 = mybir.dt.float32
BF16 = mybir.dt.bfloat16
nbf16 = ml_dtypes.bfloat16


# --------------------------------------------------------------------------
# custom DVE op: ET = Src0 * Src1 * C1 * max(Src1^4 * C0, imm2);  d += sum(ET)
#   Src0 = AT strip (mask, bf16), Src1 = e5r broadcast field (fp32)
#   C0 = e4c per-partition, C1 = e5c per-partition, imm2 = 1.0
# --------------------------------------------------------------------------
def _gat_ref(in0, in1, c0, c1, c2):
    a = np.asarray(in0, np.float32)
    e5r = np.asarray(in1, np.float32)
    P = a.shape[0]
    e4c = np.broadcast_to(np.asarray(c0, np.float32).reshape(-1, 1), (P, 1))
    e5c = np.broadcast_to(np.asarray(c1, np.float32).reshape(-1, 1), (P, 1))
    m = np.maximum((e5r ** 4) * e4c, np.float32(c2))
    body = (a * e5r * e5c * m).astype(np.float32)
    return body, body.reshape(P, -1).sum(axis=-1, keepdims=True)


def _register_gat_op():
    name = "GAT_EXPS_MASK_REDUCE"
    for o in dve_ops_mod.OPS:
        if o.name == name:
            return o
    m = maxx(sq(sq(Src1)) * C0, C2)
    spec = Spec(body=Src0 * Src1 * C1 * m, accum=add, reference=_gat_ref)
    shas = {}
    for ver in ("v3", "v4"):
        tmp = DveOpSpec(name=name, opcode=1, uops=lower(spec, ver=ver),
                        rd1_en=_has_src1(spec))
        shas[ver] = tmp.sha(ver)
    op = dve_ops_mod.DveOp(name, spec, False, shas)
    dve_ops_mod.OPS.append(op)
    dve_ops_mod.CUSTOM_DVE_SPECS[name] = spec
    dve_ops_mod._SUB_OPCODE_FOR_NAME[name] = (
        dve_ops_mod._CUSTOM_DVE_ROW_BASE + len(dve_ops_mod.OPS) - 1
    )
    assert dve_ops_mod._SUB_OPCODE_FOR_NAME[name] < 0x20
    return op


GAT_OP = _register_gat_op()


# --------------------------------------------------------------------------
# device program (SPMD; same program on all 8 cores, per-core data differs)
# --------------------------------------------------------------------------
def build_nc():
    nc = bacc.Bacc("TRN2", target_bir_lowering=False, debug=False,
                   enable_asserts=False, num_devices=NCORES)

    at_d = nc.dram_tensor("at", [N, N], BF16, kind="ExternalInput").ap()
    xt_d = nc.dram_tensor("xt", [F_IN, N], F32, kind="ExternalInput").ap()
    wcat_d = nc.dram_tensor("wcat", [F_IN, H * F_OUT + H], F32,
                            kind="ExternalInput").ap()
    w1s_d = nc.dram_tensor("w1s", [F_IN, H], F32, kind="ExternalInput").ap()
    bcol_d = nc.dram_tensor("bcol", [H, 1], F32, kind="ExternalInput").ap()
    ot_d = nc.dram_tensor("ot", [H, F_OUT, N], F32, kind="ExternalOutput").ap()

    HF = H * F_OUT  # 256

    with tile.TileContext(nc) as tc:
        with (
            tc.tile_pool(name="const", bufs=1) as cpool,
            tc.tile_pool(name="whsb", bufs=NSTRIP) as whpool,
            tc.tile_pool(name="cols", bufs=NSTRIP) as colpool,
            tc.tile_pool(name="work", bufs=3) as wpool,
            tc.tile_pool(name="et", bufs=1) as etpool,
            tc.tile_pool(name="small", bufs=4) as spool,
            tc.tile_pool(name="ps1", bufs=1, space="PSUM") as ps1,
            tc.tile_pool(name="psw", bufs=2, space="PSUM") as psw,
            tc.tile_pool(name="psot", bufs=2, space="PSUM") as psot,
        ):
            # ---- phase 0: load small tensors -------------------------------
            xt = cpool.tile([F_IN, N], F32, tag="xt")
            nc.sync.dma_start(xt[:], xt_d[:])
            wcat = cpool.tile([F_IN, HF + H], F32, tag="wcat")
            nc.sync.dma_start(wcat[:], wcat_d[:])
            w1s = cpool.tile([F_IN, H], F32, tag="w1s")
            nc.sync.dma_start(w1s[:], w1s_d[:])
            bcol = cpool.tile([H, 1], F32, tag="bcol")
            nc.sync.dma_start(bcol[:], bcol_d[:])

            # ---- phase 1: s1 rows -> e5r broadcast fields ------------------
            srows = ps1.tile([H, N], F32, tag="srows")
            nc.tensor.matmul(srows[:, 0:512], w1s[:], xt[:, 0:512],
                             start=True, stop=True)
            nc.tensor.matmul(srows[:, 512:1024], w1s[:], xt[:, 512:1024],
                             start=True, stop=True)
            e5row = cpool.tile([H, N], F32, tag="e5row")
            nc.scalar.activation(e5row[:], srows[:],
                                 mybir.ActivationFunctionType.Exp,
                                 bias=bcol[:], scale=1.0)
            e5r_bc = []
            for h in range(H):
                r0 = cpool.tile([1, N], F32, tag=f"e5row0_{h}",
                                name=f"e5row0_{h}")
                nc.sync.dma_start(r0[0:1, :], e5row[h:h + 1, :])
                t = cpool.tile([128, N], F32, tag=f"e5rbc{h}", name=f"e5rbc{h}")
                nc.gpsimd.partition_broadcast(t[:], r0[0:1, :])
                e5r_bc.append(t)

            # ---- phase 2: Wh + s2 cols per j-strip -------------------------
            wh_sb, e5c_sb, e4c_sb = [], [], []
            for js in range(NSTRIP):
                whsc = psw.tile([128, HF + H], F32, tag="whsc", name=f"whsc{js}")
                nc.tensor.matmul(whsc[:], xt[:, js * 128:(js + 1) * 128],
                                 wcat[:], start=True, stop=True)
                wh = whpool.tile([128, HF], F32, tag="wh", name=f"wh{js}")
                nc.scalar.copy(wh[:], whsc[:, 0:HF])
                wh_sb.append(wh)
                e5c = colpool.tile([128, H], F32, tag="e5c", name=f"e5c{js}")
                nc.scalar.activation(e5c[:], whsc[:, HF:HF + H],
                                     mybir.ActivationFunctionType.Exp)
                e5c_sb.append(e5c)
                e2c = spool.tile([128, H], F32, tag="e2c", name=f"e2c{js}")
                nc.vector.tensor_mul(e2c[:], e5c[:], e5c[:])
                e4c = colpool.tile([128, H], F32, tag="e4c", name=f"e4c{js}")
                nc.vector.tensor_mul(e4c[:], e2c[:], e2c[:])
                e4c_sb.append(e4c)

            # ---- phase 3: field compute (vector) ---------------------------
            ets = {}
            whps = {}
            for js in range(NSTRIP):
                at_t = wpool.tile([128, N], BF16, tag="at", name=f"at{js}")
                nc.sync.dma_start(at_t[:], at_d[js * 128:(js + 1) * 128, :])
                d4 = spool.tile([128, H], F32, tag="d4", name=f"d4{js}")
                for h in range(H):
                    et = etpool.tile([128, N], BF16, tag=f"et{js}_{h}",
                                     name=f"et{js}_{h}")
                    nc.vector._custom_dve(
                        GAT_OP, out=et[:], in0=at_t[:], in1=e5r_bc[h][:],
                        s0=e4c_sb[js][:, h:h + 1], s1=e5c_sb[js][:, h:h + 1],
                        imm2=1.0, accum_out=d4[:, h:h + 1])
                    ets[js, h] = et
                r4 = spool.tile([128, H], F32, tag="r4", name=f"r4{js}")
                nc.vector.tensor_scalar_add(r4[:], d4[:], EPS)
                nc.vector.reciprocal(r4[:], r4[:])
                for h in range(H):
                    whp = etpool.tile([128, F_OUT], BF16, tag=f"whp{js}_{h}",
                                      name=f"whp{js}_{h}")
                    nc.vector.tensor_scalar_mul(
                        whp[:], wh_sb[js][:, h * F_OUT:(h + 1) * F_OUT],
                        r4[:, h:h + 1])
                    whps[js, h] = whp

            # ---- phase 3b: per-head matmul accumulation (tensor) -----------
            ot_ps = [psot.tile([128, N], F32, tag="ot", name=f"otps{i}")
                     for i in range(2)]
            for h in range(H):
                pair, po = h // 2, (h % 2) * 64
                tp = (0, po) if po else None
                for js in range(NSTRIP):
                    for nch in range(2):
                        ns = slice(nch * 512, (nch + 1) * 512)
                        nc.tensor.matmul(
                            ot_ps[pair][po:po + 64, ns], whps[js, h][:],
                            ets[js, h][:, ns],
                            start=(js == 0), stop=(js == NSTRIP - 1),
                            tile_position=tp)

            # ---- phase 4: write out ---------------------------------------
            for pair in range(2):
                ot_sb = cpool.tile([128, N], F32, tag=f"otsb{pair}",
                                   name=f"otsb{pair}")
                nc.scalar.copy(ot_sb[:], ot_ps[pair][:])
                for hh in range(2):
                    h, po = pair * 2 + hh, hh * 64
                    nc.sync.dma_start(ot_d[h], ot_sb[po:po + 64, :])

    nc.compile()
    return nc


# --------------------------------------------------------------------------
# host-side pre/post processing
# --------------------------------------------------------------------------
def prep_in_maps(A, x, W, a_w, a_b):
    A = np.asarray(A, np.float32)
    x = np.asarray(x, np.float32)
    W = np.asarray(W, np.float32)
    a_w = np.asarray(a_w, np.float32)
    a_b = np.asarray(a_b, np.float32)

    a1, a2 = a_w[:, :F_OUT], a_w[:, F_OUT:]
    # w1s[f,h] = 0.2 * sum_o W[h,f,o] a1[h,o];  likewise w2 for the columns
    w1s = (NEG_SLOPE * np.einsum("hfo,ho->fh", W, a1)).astype(np.float32)
    w2s = (NEG_SLOPE * np.einsum("hfo,ho->fh", W, a2)).astype(np.float32)
    w4 = W.transpose(1, 0, 2).reshape(F_IN, H * F_OUT)  # [f, h*F_OUT+o]
    wcat = np.concatenate([w4, w2s], axis=1).astype(np.float32)
    bcol = (NEG_SLOPE * a_b).reshape(H, 1).astype(np.float32)

    in_maps = []
    for c in range(NCORES):
        in_maps.append({
            "at": np.ascontiguousarray(A[c].T).astype(nbf16),
            "xt": np.ascontiguousarray(x[c].T).astype(np.float32),
            "wcat": wcat,
            "w1s": w1s,
            "bcol": bcol,
        })
    return in_maps


def postprocess(results):
    out = np.empty((B, N, H * F_OUT), np.float32)
    for c in range(NCORES):
        ot = results[c]["ot"]  # [H, F_OUT, N]
        out[c] = ot.transpose(2, 0, 1).reshape(N, H * F_OUT)
    return out


_NC_CACHE = None


def get_nc():
    global _NC_CACHE
    if _NC_CACHE is None:
        _NC_CACHE = build_nc()
    return _NC_CACHE


def kernel(A, x, W, a_w, a_b):
    nc = get_nc()
    in_maps = prep_in_maps(A, x, W, a_w, a_b)
    res = run_bass_kernel_spmd(nc, in_maps, core_ids=list(range(NCORES)))
    return postprocess(res.results)


# revision 16
# speedup vs baseline: 1448.5326x; 1448.5326x over previous
"""MultiHeadGAT Bass kernel for Trainium2 (8 NeuronCores, batch-parallel).

Math (per batch b, head h):
  Wh = x @ W[h]                      (N, F_OUT)
  s1_i = Wh @ a1, s2_j = Wh @ a2     (N,)
  z[i,j] = s1_i + s2_j + ab
  exps = exp(leaky_relu(z, 0.2)) * A
  attn[i,j] = exps[i,j] / (sum_i' exps[i',j] + eps)    (softmax over dim i!)
  out = attn @ Wh; concat heads

Key identity: exp(leaky(z)) = exp(0.2 z) * max(exp(0.8 z), 1), and both
exponentials are rank-1 separable over (i, j).  With
  e5r_i = exp(0.2 (s1_i + ab)),  e5c_j = exp(0.2 s2_j),  e4c_j = exp(0.8 s2_j)
the masked field in transposed layout (j on partitions) is
  ET[j,i] = AT[j,i] * e5c_j * e5r_i * max(e5r_i^4 * e4c_j, 1)
which one custom DVE op computes per 128-row strip, bf16 out, with the
column-softmax denominator d_j = sum_i ET[j,i] accumulated for free.
TensorE then computes outT[o,i] += (Wh[j,o]/(d_j+eps)) . ET[j,i].
"""

import numpy as np
import ml_dtypes
from operator import add

import concourse.bass as bass
import concourse.bacc as bacc
import concourse.mybir as mybir
import concourse.tile as tile
import concourse.dve_ops as dve_ops_mod
from concourse.dve_spec import Spec, Src0, Src1, C0, C1, C2, sq, maxx, lower, _has_src1
from concourse.dve_uop import DveOpSpec
from concourse.bass_utils import run_bass_kernel_spmd

B, N, F_IN, F_OUT, H = 8, 1024, 128, 64, 4
EPS = 1e-7
NEG_SLOPE = 0.2
NCORES = 8
NSTRIP = N // 128  # 8 j-strips per core

F32 = mybir.dt.float32
BF16 = mybir.dt.bfloat16
nbf16 = ml_dtypes.bfloat16


# --------------------------------------------------------------------------
# custom DVE op: ET = Src0 * Src1 * C1 * max(Src1^4 * C0, imm2);  d += sum(ET)
#   Src0 = AT strip (mask, bf16), Src1 = e5r broadcast field (fp32)
#   C0 = e4c per-partition, C1 = e5c per-partition, imm2 = 1.0
# --------------------------------------------------------------------------
def _gat_ref(in0, in1, c0, c1, c2):
    a = np.asarray(in0, np.float32)
    e5r = np.asarray(in1, np.float32)
    P = a.shape[0]
    e4c = np.broadcast_to(np.asarray(c0, np.float32).reshape(-1, 1), (P, 1))
    e5c = np.broadcast_to(np.asarray(c1, np.float32).reshape(-1, 1), (P, 1))
    m = np.maximum((e5r ** 4) * e4c, np.float32(c2))
    body = (a * e5r * e5c * m).astype(np.float32)
    return body, body.reshape(P, -1).sum(axis=-1, keepdims=True)


def _register_gat_op():
    name = "GAT_EXPS_MASK_REDUCE"
    for o in dve_ops_mod.OPS:
        if o.name == name:
            return o
    m = maxx(sq(sq(Src1)) * C0, C2)
    spec = Spec(body=Src0 * Src1 * C1 * m, accum=add, reference=_gat_ref)
    shas = {}
    for ver in ("v3", "v4"):
        tmp = DveOpSpec(name=name, opcode=1, uops=lower(spec, ver=ver),
                        rd1_en=_has_src1(spec))
        shas[ver] = tmp.sha(ver)
    op = dve_ops_mod.DveOp(name, spec, False, shas)
    dve_ops_mod.OPS.append(op)
    dve_ops_mod.CUSTOM_DVE_SPECS[name] = spec
    dve_ops_mod._SUB_OPCODE_FOR_NAME[name] = (
        dve_ops_mod._CUSTOM_DVE_ROW_BASE + len(dve_ops_mod.OPS) - 1
    )
    assert dve_ops_mod._SUB_OPCODE_FOR_NAME[name] < 0x20
    return op


GAT_OP = _register_gat_op()


# --------------------------------------------------------------------------
# device program (SPMD; same program on all 8 cores, per-core data differs)
# --------------------------------------------------------------------------
def build_nc():
    nc = bacc.Bacc("TRN2", target_bir_lowering=False, debug=False,
                   enable_asserts=False, num_devices=NCORES)

    at_d = nc.dram_tensor("at", [N, N], BF16, kind="ExternalInput").ap()
    xt_d = nc.dram_tensor("xt", [F_IN, N], F32, kind="ExternalInput").ap()
    wcat_d = nc.dram_tensor("wcat", [F_IN, H * F_OUT + H], F32,
                            kind="ExternalInput").ap()
    w1s_d = nc.dram_tensor("w1s", [F_IN, H], F32, kind="ExternalInput").ap()
    bcol_d = nc.dram_tensor("bcol", [1, H], F32, kind="ExternalInput").ap()
    ot_d = nc.dram_tensor("ot", [H, F_OUT, N], F32, kind="ExternalOutput").ap()

    HF = H * F_OUT  # 256

    with tile.TileContext(nc) as tc:
        with (
            tc.tile_pool(name="const", bufs=1) as cpool,
            tc.tile_pool(name="whsb", bufs=NSTRIP) as whpool,
            tc.tile_pool(name="cols", bufs=NSTRIP) as colpool,
            tc.tile_pool(name="work", bufs=3) as wpool,
            tc.tile_pool(name="et", bufs=1) as etpool,
            tc.tile_pool(name="small", bufs=4) as spool,
            tc.tile_pool(name="ps1", bufs=1, space="PSUM") as ps1,
            tc.tile_pool(name="psw", bufs=2, space="PSUM") as psw,
            tc.tile_pool(name="psot", bufs=2, space="PSUM") as psot,
        ):
            # ---- phase 0: load small tensors -------------------------------
            xt = cpool.tile([F_IN, N], F32, tag="xt")
            nc.sync.dma_start(xt[:], xt_d[:])
            wcat = cpool.tile([F_IN, HF + H], F32, tag="wcat")
            nc.sync.dma_start(wcat[:], wcat_d[:])
            w1s = cpool.tile([F_IN, H], F32, tag="w1s")
            nc.sync.dma_start(w1s[:], w1s_d[:])
            bcol = cpool.tile([1, H], F32, tag="bcol")
            nc.sync.dma_start(bcol[:], bcol_d[:])

            # preload the Exp ACT table immediately (overlaps input DMAs)
            warm = cpool.tile([1, 8], F32, tag="warm")
            nc.vector.memset(warm[:], 0.0)
            nc.scalar.activation(warm[:], warm[:],
                                 mybir.ActivationFunctionType.Exp)

            # ---- phase 1: s1 rows -> e5r broadcast fields ------------------
            # per-head m=1 matmuls so each s1 row lands on partition 0
            e5r_bc = []
            for h in range(H):
                srow = ps1.tile([1, N], F32, tag="srow", name=f"srow{h}")
                nc.tensor.matmul(srow[0:1, 0:512], w1s[:, h:h + 1],
                                 xt[:, 0:512], start=True, stop=True)
                nc.tensor.matmul(srow[0:1, 512:1024], w1s[:, h:h + 1],
                                 xt[:, 512:1024], start=True, stop=True)
                r0 = cpool.tile([1, N], F32, tag=f"e5row0_{h}",
                                name=f"e5row0_{h}", padded_shape=[128, N])
                nc.scalar.activation(r0[0:1, :], srow[0:1, :],
                                     mybir.ActivationFunctionType.Exp,
                                     bias=bcol[0:1, h:h + 1], scale=1.0)
                t = cpool.tile([128, N], F32, tag=f"e5rbc{h}", name=f"e5rbc{h}")
                nc.gpsimd.partition_broadcast(t[:], r0[0:1, :])
                e5r_bc.append(t)

            # ---- phase 2: Wh + s2 cols per j-strip -------------------------
            wh_sb, e5c_sb, e4c_sb = [], [], []
            for js in range(NSTRIP):
                whsc = psw.tile([128, HF + H], F32, tag="whsc", name=f"whsc{js}")
                nc.tensor.matmul(whsc[:], xt[:, js * 128:(js + 1) * 128],
                                 wcat[:], start=True, stop=True)
                wh = whpool.tile([128, HF], F32, tag="wh", name=f"wh{js}")
                nc.scalar.copy(wh[:], whsc[:, 0:HF])
                wh_sb.append(wh)
                e5c = colpool.tile([128, H], F32, tag="e5c", name=f"e5c{js}")
                nc.scalar.activation(e5c[:], whsc[:, HF:HF + H],
                                     mybir.ActivationFunctionType.Exp)
                e5c_sb.append(e5c)
                e2c = spool.tile([128, H], F32, tag="e2c", name=f"e2c{js}")
                nc.vector.tensor_mul(e2c[:], e5c[:], e5c[:])
                e4c = colpool.tile([128, H], F32, tag="e4c", name=f"e4c{js}")
                nc.vector.tensor_mul(e4c[:], e2c[:], e2c[:])
                e4c_sb.append(e4c)

            # ---- phase 3: field compute (vector), head-major ---------------
            at_sb = []
            for js in range(NSTRIP):
                at_t = etpool.tile([128, N], BF16, tag=f"at{js}",
                                   name=f"at{js}")
                nc.sync.dma_start(at_t[:], at_d[js * 128:(js + 1) * 128, :])
                at_sb.append(at_t)

            ot_ps = [psot.tile([128, N], F32, tag="ot", name=f"otps{i}")
                     for i in range(2)]
            for hi, h in enumerate([0, 2, 1, 3]):
                pair, po = h // 2, (h % 2) * 64
                tp = (0, po) if po else None
                last = hi >= 2  # 3rd+4th processed heads: per-strip recip -> PE trails field
                dh = etpool.tile([128, NSTRIP], F32, tag=f"dh{h}",
                                 name=f"dh{h}")
                rh = etpool.tile([128, NSTRIP], F32, tag=f"rh{h}",
                                 name=f"rh{h}")
                ets = []
                for js in range(NSTRIP):
                    et = etpool.tile([128, N], BF16, tag=f"et{h}_{js}",
                                     name=f"et{h}_{js}")
                    nc.vector._custom_dve(
                        GAT_OP, out=et[:], in0=at_sb[js][:], in1=e5r_bc[h][:],
                        s0=e4c_sb[js][:, h:h + 1], s1=e5c_sb[js][:, h:h + 1],
                        imm2=1.0, accum_out=dh[:, js:js + 1])
                    ets.append(et)
                    if last:
                        # per-strip recip so PE trails by one strip, not a head
                        nc.vector.tensor_scalar_add(rh[:, js:js + 1],
                                                    dh[:, js:js + 1], EPS)
                        nc.vector.reciprocal(rh[:, js:js + 1], rh[:, js:js + 1])
                if not last:
                    nc.vector.tensor_scalar_add(rh[:], dh[:], EPS)
                    nc.vector.reciprocal(rh[:], rh[:])
                for js in range(NSTRIP):
                    whp = etpool.tile([128, F_OUT], BF16, tag=f"whp{h}_{js}",
                                      name=f"whp{h}_{js}")
                    nc.scalar.mul(whp[:],
                                  wh_sb[js][:, h * F_OUT:(h + 1) * F_OUT],
                                  rh[:, js:js + 1])
                    for nch in range(2):
                        ns = slice(nch * 512, (nch + 1) * 512)
                        nc.tensor.matmul(
                            ot_ps[pair][po:po + 64, ns], whp[:],
                            ets[js][:, ns],
                            start=(js == 0), stop=(js == NSTRIP - 1),
                            tile_position=tp)

            # ---- phase 4: write out ---------------------------------------
            for pair in range(2):
                ot_sb = cpool.tile([128, N], F32, tag=f"otsb{pair}",
                                   name=f"otsb{pair}")
                nc.scalar.copy(ot_sb[:], ot_ps[pair][:])
                for hh in range(2):
                    h, po = pair * 2 + hh, hh * 64
                    nc.sync.dma_start(ot_d[h], ot_sb[po:po + 64, :])

    nc.compile()
    return nc


# --------------------------------------------------------------------------
# host-side pre/post processing
# --------------------------------------------------------------------------
def prep_in_maps(A, x, W, a_w, a_b):
    A = np.asarray(A, np.float32)
    x = np.asarray(x, np.float32)
    W = np.asarray(W, np.float32)
    a_w = np.asarray(a_w, np.float32)
    a_b = np.asarray(a_b, np.float32)

    a1, a2 = a_w[:, :F_OUT], a_w[:, F_OUT:]
    # w1s[f,h] = 0.2 * sum_o W[h,f,o] a1[h,o];  likewise w2 for the columns
    w1s = (NEG_SLOPE * np.einsum("hfo,ho->fh", W, a1)).astype(np.float32)
    w2s = (NEG_SLOPE * np.einsum("hfo,ho->fh", W, a2)).astype(np.float32)
    w4 = W.transpose(1, 0, 2).reshape(F_IN, H * F_OUT)  # [f, h*F_OUT+o]
    wcat = np.concatenate([w4, w2s], axis=1).astype(np.float32)
    bcol = (NEG_SLOPE * a_b).reshape(1, H).astype(np.float32)

    in_maps = []
    for c in range(NCORES):
        in_maps.append({
            "at": np.ascontiguousarray(A[c].T).astype(nbf16),
            "xt": np.ascontiguousarray(x[c].T).astype(np.float32),
            "wcat": wcat,
            "w1s": w1s,
            "bcol": bcol,
        })
    return in_maps


def postprocess(results):
    out = np.empty((B, N, H * F_OUT), np.float32)
    for c in range(NCORES):
        ot = results[c]["ot"]  # [H, F_OUT, N]
        out[c] = ot.transpose(2, 0, 1).reshape(N, H * F_OUT)
    return out


_NC_CACHE = None


def get_nc():
    global _NC_CACHE
    if _NC_CACHE is None:
        _NC_CACHE = build_nc()
    return _NC_CACHE


def kernel(A, x, W, a_w, a_b):
    nc = get_nc()
    in_maps = prep_in_maps(A, x, W, a_w, a_b)
    res = run_bass_kernel_spmd(nc, in_maps, core_ids=list(range(NCORES)))
    return postprocess(res.results)
